# revision 1
# baseline (speedup 1.0000x reference)
"""Causal multi-head attention forward on 8 Trainium2 NeuronCores.

Problem: nn_CoreAttention (SQ=SK=2048, B=2, NP=16 heads, HN=128, fp32).

Sharding: the 32 (batch, head) pairs are split 4 per core (tensor-parallel
over heads, data-parallel over batch). No collectives needed.

Per (b, n) pair the kernel computes, in transposed score orientation:
    scoresT[sk, sq] = (K Q^T) / sqrt(HN)      (PE matmul, hn contracted)
    expT = exp(scoresT + additive_mask)       (ScalarE, fused scale, fp16 out)
    ctx_aug[sq, hn+1] = expT^T @ [V | 1]      (PE matmul, sk contracted;
                                               col hn holds the softmax denom)
    ctx = ctx_aug[:, :hn] * 1/ctx_aug[:, hn]  (DVE reciprocal + scale)

The block schedule (which 128x128 score blocks are skipped / masked) is
derived from the actual attention_mask at build time, so any mask pattern
produces a correct (if differently-sized) kernel. The causal mask gives the
standard lower-triangular schedule with one unique triangular additive tile.
"""

import math
import numpy as np
from contextlib import ExitStack

import concourse.bacc as bacc
import concourse.tile as tile
from concourse import mybir

SQ, SK, B, NP, HN = 2048, 2048, 2, 16, 128
N_CORES = 8
SLOTS_PER_CORE = 4  # (b, n) pairs per core
P = 128             # partition dim / block size
CHUNK = 256         # sq chunk width for QK matmuls (fp16/fp32r full rate)
import os
GROUP = int(os.environ.get("ATT_GROUP", "4"))
SC_BUFS = int(os.environ.get("ATT_SC_BUFS", "3"))
CX_BUFS = int(os.environ.get("ATT_CX_BUFS", "2"))
E_BUFS = int(os.environ.get("ATT_E_BUFS", "6"))
N_SQ_TILES = SQ // P        # 16
N_SK_TILES = SK // P        # 16
N_CHUNKS = SQ // CHUNK      # 8
NEG = -60000.0              # additive mask value; exp -> exactly 0

QK_MODE = os.environ.get("ATT_QK_MODE", "fp32r")  # "fp32r" | "fp16" | "bf16x3"

F32 = mybir.dt.float32
F32R = mybir.dt.float32r
F16 = mybir.dt.float16
BF16 = mybir.dt.bfloat16

SKIP, FULL, PARTIAL = 0, 1, 2


def _block_schedule(mask_b: np.ndarray):
    """Classify each 128x128 (sk_tile j, sq_tile i) block of one batch's mask.

    Returns (status[j][i], tiles) where tiles maps uid -> additive fp32
    [128(sk), 128(sq)] tile (transposed into scoresT orientation).
    """
    m4 = mask_b.reshape(N_SQ_TILES, P, N_SK_TILES, P)
    alls = m4.all(axis=(1, 3))  # [i, j]
    anys = m4.any(axis=(1, 3))
    status = np.zeros((N_SK_TILES, N_SQ_TILES), dtype=np.int64)
    tiles: dict[bytes, int] = {}
    uniq: list[np.ndarray] = []
    uid_of: dict[tuple[int, int], int] = {}
    for j in range(N_SK_TILES):
        for i in range(N_SQ_TILES):
            if alls[i, j]:
                status[j, i] = SKIP
            elif not anys[i, j]:
                status[j, i] = FULL
            else:
                status[j, i] = PARTIAL
                t = np.where(m4[i, :, j, :].T, np.float32(NEG), np.float32(0.0))
                key = t.tobytes()
                if key not in tiles:
                    tiles[key] = len(uniq)
                    uniq.append(t)
                uid_of[(j, i)] = tiles[key]
    return status, uniq, uid_of


def _build_program(schedules, n_mask_tiles):
    """Build the SPMD bass program. schedules[slot] = (status, uid_of)."""
    nc = bacc.Bacc()

    qT_d = nc.declare_dram_parameter("qT", [SLOTS_PER_CORE, P, SQ], F32, isOutput=False)
    kT_d = nc.declare_dram_parameter("kT", [SLOTS_PER_CORE, P, SK], F32, isOutput=False)
    v_d = nc.declare_dram_parameter(
        "v_aug", [SLOTS_PER_CORE, P, N_SK_TILES, HN + 1], F16, isOutput=False
    )
    mt_d = None
    if n_mask_tiles:
        mt_d = nc.declare_dram_parameter(
            "mask_tiles", [P, n_mask_tiles * P], F32, isOutput=False
        )
    out_d = nc.declare_dram_parameter(
        "out", [SLOTS_PER_CORE, N_SQ_TILES, P, HN], F32, isOutput=True
    )

    inv_norm = 1.0 / math.sqrt(HN)

    with tile.TileContext(nc) as tc, ExitStack() as ctx:
        qk_pool = ctx.enter_context(tc.tile_pool(name="qk", bufs=2))
        qkr_pool = ctx.enter_context(tc.tile_pool(name="qkr", bufs=2))
        v_pool = ctx.enter_context(tc.tile_pool(name="v", bufs=2))
        m_pool = ctx.enter_context(tc.tile_pool(name="m", bufs=1))
        e_pool = ctx.enter_context(tc.tile_pool(name="e", bufs=E_BUFS))
        o_pool = ctx.enter_context(tc.tile_pool(name="o", bufs=4))
        r_pool = ctx.enter_context(tc.tile_pool(name="r", bufs=4))
        sc_ps = ctx.enter_context(tc.tile_pool(name="sc", bufs=SC_BUFS, space="PSUM"))
        cx_ps = ctx.enter_context(tc.tile_pool(name="cx", bufs=CX_BUFS, space="PSUM"))

        mask_sb = None
        if n_mask_tiles:
            mask_sb = m_pool.tile([P, n_mask_tiles * P], F32, tag="mask")
            nc.sync.dma_start(mask_sb[:], mt_d[:])

        for slot in range(SLOTS_PER_CORE):
            status, uid_of = schedules[slot]
            if QK_MODE == "fp32r":
                qT32 = qk_pool.tile([P, SQ], F32, tag="q32")
                nc.sync.dma_start(qT32[:], qT_d[slot])
                kT32 = qk_pool.tile([P, SK], F32, tag="k32")
                nc.sync.dma_start(kT32[:], kT_d[slot])
                qT = qkr_pool.tile([P, SQ], F32R, tag="qr")
                nc.vector.tensor_copy(qT[:], qT32[:])
                kT = qkr_pool.tile([P, SK], F32R, tag="kr")
                nc.vector.tensor_copy(kT[:], kT32[:])
            elif QK_MODE == "fp16":
                # host supplies fp32; cast via DVE to fp16
                qT32 = qk_pool.tile([P, SQ], F32, tag="q32")
                nc.sync.dma_start(qT32[:], qT_d[slot])
                kT32 = qk_pool.tile([P, SK], F32, tag="k32")
                nc.sync.dma_start(kT32[:], kT_d[slot])
                qT = qkr_pool.tile([P, SQ], F16, tag="qr")
                nc.vector.tensor_copy(qT[:], qT32[:])
                kT = qkr_pool.tile([P, SK], F16, tag="kr")
                nc.vector.tensor_copy(kT[:], kT32[:])
            else:  # bf16x3
                qT32 = qk_pool.tile([P, SQ], F32, tag="q32")
                nc.sync.dma_start(qT32[:], qT_d[slot])
                kT32 = qk_pool.tile([P, SK], F32, tag="k32")
                nc.sync.dma_start(kT32[:], kT_d[slot])
                qhi = qkr_pool.tile([P, SQ], BF16, tag="qhi")
                nc.vector.tensor_copy(qhi[:], qT32[:])
                khi = qkr_pool.tile([P, SK], BF16, tag="khi")
                nc.vector.tensor_copy(khi[:], kT32[:])
                qhi32 = qkr_pool.tile([P, SQ], F32, tag="qhi32")
                nc.vector.tensor_copy(qhi32[:], qhi[:])
                khi32 = qkr_pool.tile([P, SK], F32, tag="khi32")
                nc.vector.tensor_copy(khi32[:], khi[:])
                qlo = qkr_pool.tile([P, SQ], BF16, tag="qlo")
                nc.vector.tensor_sub(qlo[:], qT32[:], qhi32[:])
                klo = qkr_pool.tile([P, SK], BF16, tag="klo")
                nc.vector.tensor_sub(klo[:], kT32[:], khi32[:])

            v_sb = v_pool.tile([P, N_SK_TILES * (HN + 1)], F16, tag="v")
            nc.sync.dma_start(
                v_sb[:], v_d[slot].rearrange("p t c -> p (t c)")
            )
            for ci in range(N_CHUNKS):
                i_tiles = [
                    i
                    for i in range(ci * CHUNK // P, (ci + 1) * CHUNK // P)
                    if any(status[j, i] != SKIP for j in range(N_SK_TILES))
                ]
                if not i_tiles:
                    continue
                # sk tiles needed for this sq chunk
                js = [
                    j
                    for j in range(N_SK_TILES)
                    if any(status[j, i] != SKIP for i in i_tiles)
                ]
                c0 = ci * CHUNK

                # group j's into PSUM group tiles of up to GROUP blocks
                exp_tiles: dict[int, tuple] = {}  # j -> (expT tile, col offset)
                for g0 in range(0, len(js), GROUP):
                    gjs = js[g0 : g0 + GROUP]
                    width = len(gjs) * CHUNK
                    sc = sc_ps.tile([P, GROUP * CHUNK], F32, tag="scores")
                    for k, j in enumerate(gjs):
                        co = k * CHUNK
                        if QK_MODE == "bf16x3":
                            nc.tensor.matmul(
                                sc[:, co : co + CHUNK],
                                khi[:, j * P : (j + 1) * P],
                                qhi[:, c0 : c0 + CHUNK],
                                start=True, stop=False,
                            )
                            nc.tensor.matmul(
                                sc[:, co : co + CHUNK],
                                khi[:, j * P : (j + 1) * P],
                                qlo[:, c0 : c0 + CHUNK],
                                start=False, stop=False,
                            )
                            nc.tensor.matmul(
                                sc[:, co : co + CHUNK],
                                klo[:, j * P : (j + 1) * P],
                                qhi[:, c0 : c0 + CHUNK],
                                start=False, stop=True,
                            )
                        else:
                            nc.tensor.matmul(
                                sc[:, co : co + CHUNK],
                                kT[:, j * P : (j + 1) * P],
                                qT[:, c0 : c0 + CHUNK],
                                start=True, stop=True,
                            )
                        # additive mask tiles for partial sub-blocks
                        for h, i in enumerate(range(ci * CHUNK // P, (ci + 1) * CHUNK // P)):
                            if status[j, i] == PARTIAL:
                                uid = uid_of[(j, i)]
                                nc.vector.tensor_add(
                                    sc[:, co + h * P : co + (h + 1) * P],
                                    sc[:, co + h * P : co + (h + 1) * P],
                                    mask_sb[:, uid * P : (uid + 1) * P],
                                )
                    et = e_pool.tile([P, GROUP * CHUNK], F16, tag="expT")
                    nc.scalar.activation(
                        et[:, :width], sc[:, :width],
                        mybir.ActivationFunctionType.Exp,
                        scale=inv_norm,
                    )
                    for k, j in enumerate(gjs):
                        exp_tiles[j] = (et, k * CHUNK)

                # PV per 128-wide sq tile of this chunk
                for ii, i in enumerate(i_tiles):
                    pv_js = [j for j in range(N_SK_TILES) if status[j, i] != SKIP]
                    cx = cx_ps.tile([P, HN + 1], F32, tag="ctx")
                    for idx, j in enumerate(pv_js):
                        et, co = exp_tiles[j]
                        icol = co + (i - ci * CHUNK // P) * P
                        nc.tensor.matmul(
                            cx[:],
                            et[:, icol : icol + P],
                            v_sb[:, j * (HN + 1) : (j + 1) * (HN + 1)],
                            start=(idx == 0),
                            stop=(idx == len(pv_js) - 1),
                        )
                    recip = r_pool.tile([P, 1], F32, tag="recip")
                    nc.vector.reciprocal(recip[:], cx[:, HN : HN + 1])
                    o_sb = o_pool.tile([P, HN], F32, tag="out")
                    nc.vector.tensor_scalar_mul(o_sb[:], cx[:, 0:HN], recip[:])
                    nc.sync.dma_start(out_d[slot, i], o_sb[:])

    nc.compile()
    return nc


_cache = {}


def _get_program(mask: np.ndarray):
    key = mask.tobytes()
    if key in _cache:
        return _cache[key]

    # schedules per batch; slots [0,1] -> b=0, [2,3] -> b=1 (same for all cores)
    scheds = []
    all_tiles: list[np.ndarray] = []
    tile_index: dict[bytes, int] = {}
    for b in range(B):
        status, uniq, uid_of = _block_schedule(np.asarray(mask[b, 0]))
        remap = {}
        for local_uid, t in enumerate(uniq):
            k = t.tobytes()
            if k not in tile_index:
                tile_index[k] = len(all_tiles)
                all_tiles.append(t)
            remap[local_uid] = tile_index[k]
        uid_of = {ji: remap[u] for ji, u in uid_of.items()}
        scheds.append((status, uid_of))

    slot_scheds = [scheds[0], scheds[0], scheds[1], scheds[1]]
    n_tiles = len(all_tiles)
    nc = _build_program(slot_scheds, n_tiles)

    if n_tiles:
        mt = np.stack(all_tiles)  # [U, 128, 128]
        mask_tiles = np.ascontiguousarray(mt.transpose(1, 0, 2)).reshape(
            P, n_tiles * P
        )
    else:
        mask_tiles = None
    _cache[key] = (nc, mask_tiles)
    return _cache[key]


def _core_slots(c):
    return [(0, 2 * c), (0, 2 * c + 1), (1, 2 * c), (1, 2 * c + 1)]


def prepare(query_layer, key_layer, value_layer, attention_mask):
    """Build (nc, in_maps). Shared by kernel() and the benchmark harness."""
    q = np.asarray(query_layer, dtype=np.float32)
    k = np.asarray(key_layer, dtype=np.float32)
    v = np.asarray(value_layer, dtype=np.float32)
    mask = np.asarray(attention_mask)

    nc, mask_tiles = _get_program(mask)

    # host layout prep
    # qT_all[b, n] = q[:, b, n, :].T  -> [B, NP, 128, SQ]
    qT_all = np.ascontiguousarray(q.transpose(1, 2, 3, 0))
    kT_all = np.ascontiguousarray(k.transpose(1, 2, 3, 0))
    # v_aug_all[b, n, p, t, c] = v[t*128+p, b, n, c], plus ones column
    v5 = v.reshape(N_SK_TILES, P, B, NP, HN).transpose(2, 3, 1, 0, 4)
    v_aug_all = np.empty((B, NP, P, N_SK_TILES, HN + 1), dtype=np.float16)
    v_aug_all[..., :HN] = v5
    v_aug_all[..., HN] = 1.0

    in_maps = []
    for c in range(N_CORES):
        slots = _core_slots(c)
        im = {
            "qT": np.ascontiguousarray(np.stack([qT_all[b, n] for b, n in slots])),
            "kT": np.ascontiguousarray(np.stack([kT_all[b, n] for b, n in slots])),
            "v_aug": np.ascontiguousarray(
                np.stack([v_aug_all[b, n] for b, n in slots])
            ),
        }
        if mask_tiles is not None:
            im["mask_tiles"] = mask_tiles
        in_maps.append(im)
    return nc, in_maps


def assemble(results):
    """Gather per-core 'out' arrays into the full [SQ, B, NP*HN] output."""
    full = np.empty((SQ, B, NP * HN), dtype=np.float32)
    for c in range(N_CORES):
        o = results[c]["out"]  # [4, 16, 128, 128]
        for s, (b, n) in enumerate(_core_slots(c)):
            full[:, b, n * HN : (n + 1) * HN] = o[s].reshape(SQ, HN)
    return full


def kernel(query_layer, key_layer, value_layer, attention_mask):
    from concourse.bass_utils import run_bass_kernel_spmd

    nc, in_maps = prepare(query_layer, key_layer, value_layer, attention_mask)
    res = run_bass_kernel_spmd(nc, in_maps, list(range(N_CORES)))
    return assemble(res.results)



# revision 3
# speedup vs baseline: 448.3255x; 448.3255x over previous
"""Causal multi-head attention forward on 8 Trainium2 NeuronCores.

Problem: nn_CoreAttention (SQ=SK=2048, B=2, NP=16 heads, HN=128, fp32).

Sharding: 32 (batch, head) pairs split 4 per core (tensor-parallel over
heads, data-parallel over batch). No collectives.

Per (b, n) pair, in transposed score orientation (sk on partitions):
    scoresT[sk, sq] = (K Q^T)                 (PE matmul, fp16 in, hn contracted)
    expT = exp(scoresT * 1/sqrt(HN) + mask)   (ScalarE, fp16 out)
    ctx_aug[sq, hn+1] = expT^T @ [V | 1]      (PE matmul, sk contracted;
                                               col hn holds the softmax denom)
    ctx = ctx_aug[:, :hn] * 1/ctx_aug[:, hn]  (DVE reciprocal + scale)

v2 structure (vs the v1 baseline):
  - q/k cast to fp16 on host; no on-device casts; FWL stays enabled.
  - sq chunks of 256 with the two 128-tiles SWAPPED (host pre-swap), so the
    diagonal j-tile's masked half is the trailing half of the chunk: the
    diagonal QK matmul streams only 128 valid cols and exp skips the rest.
  - the two triangular tiles per chunk land on 256 contiguous score cols:
    one DVE add with a single [128,256] additive tile handles all masking.
  - PV is software-pipelined one chunk behind QK so the PE never stalls on
    the exp latency of the chunk it just produced.
  - slot 0 loads q/k/v in small pieces (fast start); the last slot walks
    chunks descending so the kernel tail is the smallest chunk.
  - outputs accumulate in SBUF and leave as 4 wide DMAs per slot.
"""

import math
import numpy as np
from contextlib import ExitStack

import concourse.bacc as bacc
import concourse.tile as tile
from concourse import mybir

SQ, SK, B, NP, HN = 2048, 2048, 2, 16, 128
N_CORES = 8
SLOTS_PER_CORE = 4
P = 128
CHUNK = 256
N_CHUNKS = SQ // CHUNK      # 8
N_SK_TILES = SK // P        # 16
NEG = -60000.0

import os
GROUP = int(os.environ.get("ATT_GROUP", "6"))          # j-tiles per PSUM group
SC_BUFS = int(os.environ.get("ATT_SC_BUFS", "2"))
CX_BUFS = int(os.environ.get("ATT_CX_BUFS", "2"))
E_BUFS = int(os.environ.get("ATT_E_BUFS", "6"))

F32 = mybir.dt.float32
F16 = mybir.dt.float16


def _build_program():
    nc = bacc.Bacc()

    qT_d = nc.declare_dram_parameter("qT", [SLOTS_PER_CORE, P, SQ], F16, isOutput=False)
    kT_d = nc.declare_dram_parameter("kT", [SLOTS_PER_CORE, P, SK], F16, isOutput=False)
    v_d = nc.declare_dram_parameter(
        "v_aug", [SLOTS_PER_CORE, P, N_SK_TILES * (HN + 1)], F16, isOutput=False
    )
    tri_d = nc.declare_dram_parameter("tri", [P, 2 * P], F32, isOutput=False)
    out_d = nc.declare_dram_parameter(
        "out", [SLOTS_PER_CORE, 4, P, 4 * HN], F32, isOutput=True
    )

    inv_norm = 1.0 / math.sqrt(HN)

    with tile.TileContext(nc) as tc, ExitStack() as ctx:
        qk_pool = ctx.enter_context(tc.tile_pool(name="qk", bufs=2))
        v_pool = ctx.enter_context(tc.tile_pool(name="v", bufs=2))
        m_pool = ctx.enter_context(tc.tile_pool(name="m", bufs=1))
        e_pool = ctx.enter_context(tc.tile_pool(name="e", bufs=E_BUFS))
        o_pool = ctx.enter_context(tc.tile_pool(name="o", bufs=3))
        r_pool = ctx.enter_context(tc.tile_pool(name="r", bufs=4))
        sc_ps = ctx.enter_context(tc.tile_pool(name="sc", bufs=SC_BUFS, space="PSUM"))
        cx_ps = ctx.enter_context(tc.tile_pool(name="cx", bufs=CX_BUFS, space="PSUM"))

        tri_sb = m_pool.tile([P, 2 * P], F32, tag="tri")
        nc.sync.dma_start(tri_sb[:], tri_d[:])

        # ---- per-slot input loading -------------------------------------
        def load_slot(slot):
            """Returns (kslice, qchunk, vslice) accessor fns for this slot."""
            if slot == 0:
                # small pieces so the first matmuls start early
                kts = []
                for pc in range(4):
                    t = qk_pool.tile([P, 4 * P], F16, tag=f"k{pc}")
                    nc.sync.dma_start(t[:], kT_d[slot][:, pc * 512 : (pc + 1) * 512])
                    kts.append(t)
                qts = []
                for pc in range(N_CHUNKS):
                    t = qk_pool.tile([P, CHUNK], F16, tag=f"q{pc}")
                    nc.sync.dma_start(t[:], qT_d[slot][:, pc * CHUNK : (pc + 1) * CHUNK])
                    qts.append(t)
                vts = []
                for pc in range(4):
                    t = v_pool.tile([P, 4 * (HN + 1)], F16, tag=f"v{pc}")
                    nc.sync.dma_start(
                        t[:], v_d[slot][:, pc * 4 * (HN + 1) : (pc + 1) * 4 * (HN + 1)]
                    )
                    vts.append(t)
                kslice = lambda j: kts[j // 4][:, (j % 4) * P : (j % 4 + 1) * P]
                qchunk = lambda ci: qts[ci][:]
                vslice = lambda j: vts[j // 4][
                    :, (j % 4) * (HN + 1) : (j % 4 + 1) * (HN + 1)
                ]
            else:
                kt = qk_pool.tile([P, SK], F16, tag="k")
                nc.sync.dma_start(kt[:], kT_d[slot])
                qt = qk_pool.tile([P, SQ], F16, tag="q")
                nc.sync.dma_start(qt[:], qT_d[slot])
                vt = v_pool.tile([P, N_SK_TILES * (HN + 1)], F16, tag="v")
                nc.sync.dma_start(vt[:], v_d[slot])
                kslice = lambda j: kt[:, j * P : (j + 1) * P]
                qchunk = lambda ci: qt[:, ci * CHUNK : (ci + 1) * CHUNK]
                vslice = lambda j: vt[:, j * (HN + 1) : (j + 1) * (HN + 1)]
            return kslice, qchunk, vslice

        # ---- emit one chunk's QK + exp; return et lookup ----------------
        def emit_qk(slot_io, ci):
            kslice, qchunk, _ = slot_io
            js = list(range(2 * ci + 2))     # ascending; diagonal j last
            diag = 2 * ci + 1
            exp_tiles = {}
            for g0 in range(0, len(js), GROUP):
                gjs = js[g0 : g0 + GROUP]
                sc = sc_ps.tile([P, GROUP * CHUNK], F32, tag="scores")
                gw = 0
                for k_idx, j in enumerate(gjs):
                    co = k_idx * CHUNK
                    w = P if j == diag else CHUNK
                    nc.tensor.matmul(
                        sc[:, co : co + w], kslice(j), qchunk(ci)[:, 0:w],
                        start=True, stop=True,
                    )
                    gw = co + w
                if diag in gjs:
                    # both triangular tiles sit on 256 contiguous cols:
                    # (second half of j=2ci) ++ (first half of j=2ci+1)
                    moff = gjs.index(2 * ci) * CHUNK + P
                    nc.vector.tensor_add(
                        sc[:, moff : moff + 2 * P],
                        sc[:, moff : moff + 2 * P],
                        tri_sb[:],
                    )
                et = e_pool.tile([P, GROUP * CHUNK], F16, tag="expT")
                nc.scalar.activation(
                    et[:, :gw], sc[:, :gw],
                    mybir.ActivationFunctionType.Exp,
                    scale=inv_norm,
                )
                for k_idx, j in enumerate(gjs):
                    exp_tiles[j] = (et, k_idx * CHUNK)
            return exp_tiles

        # ---- emit one chunk's PV + normalize + (maybe) out DMA ----------
        def emit_pv(slot, slot_io, ci, exp_tiles, oq_tiles, done_quarters):
            _, _, vslice = slot_io
            for i in (2 * ci + 1, 2 * ci):   # i_hi (first half of chunk), i_lo
                off = 0 if i == 2 * ci + 1 else P
                pv_js = list(range(i + 1))
                cx = cx_ps.tile([P, HN + 1], F32, tag="ctx")
                for idx, j in enumerate(pv_js):
                    et, co = exp_tiles[j]
                    nc.tensor.matmul(
                        cx[:], et[:, co + off : co + off + P], vslice(j),
                        start=(idx == 0), stop=(idx == len(pv_js) - 1),
                    )
                recip = r_pool.tile([P, 1], F32, tag="recip")
                nc.vector.reciprocal(recip[:], cx[:, HN : HN + 1])
                qt_idx = i // 4
                if qt_idx not in oq_tiles:
                    oq_tiles[qt_idx] = o_pool.tile(
                        [P, 4 * HN], F32, tag="oq", name="oq"
                    )
                ot = oq_tiles[qt_idx]
                col = (i % 4) * HN
                nc.vector.tensor_scalar_mul(
                    ot[:, col : col + HN], cx[:, 0:HN], recip[:]
                )
            # quarter complete -> ship it
            qt_idx = (2 * ci) // 4
            key = (ci // 2, ci % 2)
            done_quarters.setdefault(qt_idx, set()).add(ci)
            if len(done_quarters[qt_idx]) == 2:
                nc.sync.dma_start(out_d[slot, qt_idx], oq_tiles[qt_idx][:])

        # ---- main schedule: PV pipelined one chunk behind QK ------------
        pending = None  # (slot, slot_io, ci, exp_tiles, oq_tiles, done_quarters)
        slot_state = {}
        for slot in range(SLOTS_PER_CORE):
            slot_io = load_slot(slot)
            order = range(N_CHUNKS) if slot < SLOTS_PER_CORE - 1 else \
                range(N_CHUNKS - 1, -1, -1)
            slot_state[slot] = ({}, {})  # oq_tiles, done_quarters
            for ci in order:
                exp_tiles = emit_qk(slot_io, ci)
                if pending is not None:
                    emit_pv(*pending)
                oq, dq = slot_state[slot]
                pending = (slot, slot_io, ci, exp_tiles, oq, dq)
        emit_pv(*pending)

    nc.compile()
    return nc


_cache = {}


def _get_program(mask: np.ndarray):
    # this kernel is specialized to the standard causal mask
    m = np.asarray(mask)
    causal = np.triu(np.ones((SQ, SK), dtype=bool), k=1)
    for b in range(B):
        if not np.array_equal(m[b, 0], causal):
            raise ValueError("kernel specialized to causal attention mask")
    if "nc" not in _cache:
        _cache["nc"] = _build_program()
    return _cache["nc"]


def _core_slots(c):
    return [(0, 2 * c), (0, 2 * c + 1), (1, 2 * c), (1, 2 * c + 1)]


def prepare(query_layer, key_layer, value_layer, attention_mask):
    q = np.asarray(query_layer)
    k = np.asarray(key_layer)
    v = np.asarray(value_layer)
    nc = _get_program(np.asarray(attention_mask))

    # qT with the two 128-col tiles of each 256 chunk swapped:
    # sbuf layout col (256*ci + [0..255]) = sq (256*ci + [128..255, 0..127])
    q16 = q.astype(np.float16)                      # [SQ, B, NP, HN]
    qv = q16.reshape(N_CHUNKS, 2, P, B, NP, HN)[:, ::-1]   # swap tile pairs
    qT_all = np.ascontiguousarray(qv.transpose(3, 4, 5, 0, 1, 2)).reshape(
        B, NP, HN, SQ
    )
    k16 = k.astype(np.float16)
    kT_all = np.ascontiguousarray(k16.transpose(1, 2, 3, 0))  # [B, NP, HN, SK]

    v5 = v.reshape(N_SK_TILES, P, B, NP, HN).transpose(2, 3, 1, 0, 4)
    v_aug_all = np.empty((B, NP, P, N_SK_TILES, HN + 1), dtype=np.float16)
    v_aug_all[..., :HN] = v5
    v_aug_all[..., HN] = 1.0
    v_aug_all = v_aug_all.reshape(B, NP, P, N_SK_TILES * (HN + 1))

    # additive triangular tile (scoresT orientation): mask where sq < sk
    tri1 = np.where(
        np.arange(P)[None, :] < np.arange(P)[:, None], np.float32(NEG), np.float32(0)
    )
    tri = np.concatenate([tri1, tri1], axis=1)      # [128, 256]

    in_maps = []
    for c in range(N_CORES):
        slots = _core_slots(c)
        im = {
            "qT": np.ascontiguousarray(np.stack([qT_all[b, n] for b, n in slots])),
            "kT": np.ascontiguousarray(np.stack([kT_all[b, n] for b, n in slots])),
            "v_aug": np.ascontiguousarray(
                np.stack([v_aug_all[b, n] for b, n in slots])
            ),
            "tri": tri,
        }
        in_maps.append(im)
    return nc, in_maps


def assemble(results):
    """Gather per-core 'out' arrays into the full [SQ, B, NP*HN] output."""
    full = np.empty((SQ, B, NP * HN), dtype=np.float32)
    for c in range(N_CORES):
        o = results[c]["out"]  # [4, 4, 128, 512]
        for s, (b, n) in enumerate(_core_slots(c)):
            ctx = (
                o[s].reshape(4, P, 4, HN).transpose(0, 2, 1, 3).reshape(SQ, HN)
            )
            full[:, b, n * HN : (n + 1) * HN] = ctx
    return full


def kernel(query_layer, key_layer, value_layer, attention_mask):
    from concourse.bass_utils import run_bass_kernel_spmd

    nc, in_maps = prepare(query_layer, key_layer, value_layer, attention_mask)
    res = run_bass_kernel_spmd(nc, in_maps, list(range(N_CORES)))
    return assemble(res.results)


# revision 6
# speedup vs baseline: 463.9681x; 1.0349x over previous
"""Causal multi-head attention forward on 8 Trainium2 NeuronCores.

Problem: nn_CoreAttention (SQ=SK=2048, B=2, NP=16 heads, HN=128, fp32).

Sharding: 32 (batch, head) pairs split 4 per core (tensor-parallel over
heads, data-parallel over batch). No collectives.

Per (b, n) pair, in transposed score orientation (sk on partitions):
    scoresT[sk, sq] = (K Q^T)                 (PE matmul, fp16 in, hn contracted)
    expT = exp(scoresT * 1/sqrt(HN) + mask)   (ScalarE, fp16 out)
    ctx_aug[sq, hn+1] = expT^T @ [V | 1]      (PE matmul, sk contracted;
                                               col hn holds the softmax denom)
    ctx = ctx_aug[:, :hn] * 1/ctx_aug[:, hn]  (DVE reciprocal + scale)

v2 structure (vs the v1 baseline):
  - q/k cast to fp16 on host; no on-device casts; FWL stays enabled.
  - sq chunks of 256 with the two 128-tiles SWAPPED (host pre-swap), so the
    diagonal j-tile's masked half is the trailing half of the chunk: the
    diagonal QK matmul streams only 128 valid cols and exp skips the rest.
  - the two triangular tiles per chunk land on 256 contiguous score cols:
    one DVE add with a single [128,256] additive tile handles all masking.
  - PV is software-pipelined one chunk behind QK so the PE never stalls on
    the exp latency of the chunk it just produced.
  - slot 0 loads q/k/v in small pieces (fast start); the last slot walks
    chunks descending so the kernel tail is the smallest chunk.
  - outputs accumulate in SBUF and leave as 4 wide DMAs per slot.
"""

import math
import numpy as np
from contextlib import ExitStack

import concourse.bacc as bacc
import concourse.tile as tile
from concourse import mybir

SQ, SK, B, NP, HN = 2048, 2048, 2, 16, 128
N_CORES = 8
SLOTS_PER_CORE = 4
P = 128
CHUNK = 256
N_CHUNKS = SQ // CHUNK      # 8
N_SK_TILES = SK // P        # 16
NEG = -60000.0

import os
GROUP = int(os.environ.get("ATT_GROUP", "6"))          # j-tiles per PSUM group
SC_BUFS = int(os.environ.get("ATT_SC_BUFS", "2"))
CX_BUFS = int(os.environ.get("ATT_CX_BUFS", "2"))
E_BUFS = int(os.environ.get("ATT_E_BUFS", "6"))

F32 = mybir.dt.float32
F16 = mybir.dt.float16


def _build_program():
    nc = bacc.Bacc()

    qT_d = nc.declare_dram_parameter("qT", [SLOTS_PER_CORE, P, SQ], F16, isOutput=False)
    kT_d = nc.declare_dram_parameter("kT", [SLOTS_PER_CORE, P, SK], F16, isOutput=False)
    v_d = nc.declare_dram_parameter(
        "v_aug", [SLOTS_PER_CORE, P, N_SK_TILES * (HN + 1)], F16, isOutput=False
    )
    tri_d = nc.declare_dram_parameter("tri", [P, 2 * P], F32, isOutput=False)
    out_d = nc.declare_dram_parameter(
        "out", [SLOTS_PER_CORE, 4, P, 4 * HN], F32, isOutput=True
    )

    inv_norm = 1.0 / math.sqrt(HN)

    with tile.TileContext(nc) as tc, ExitStack() as ctx:
        qk_pool = ctx.enter_context(tc.tile_pool(name="qk", bufs=2))
        v_pool = ctx.enter_context(tc.tile_pool(name="v", bufs=2))
        m_pool = ctx.enter_context(tc.tile_pool(name="m", bufs=1))
        e_pool = ctx.enter_context(tc.tile_pool(name="e", bufs=E_BUFS))
        o_pool = ctx.enter_context(tc.tile_pool(name="o", bufs=3))
        r_pool = ctx.enter_context(tc.tile_pool(name="r", bufs=4))
        sc_ps = ctx.enter_context(tc.tile_pool(name="sc", bufs=SC_BUFS, space="PSUM"))
        cx_ps = ctx.enter_context(tc.tile_pool(name="cx", bufs=CX_BUFS, space="PSUM"))

        tri_sb = m_pool.tile([P, 2 * P], F32, tag="tri")
        nc.sync.dma_start(tri_sb[:], tri_d[:])

        # ---- per-slot input loading -------------------------------------
        def load_slot(slot):
            """Returns (kslice, qchunk, vslice) accessor fns for this slot."""
            if slot == 0:
                # pieces, issued in first-use order (chunks run descending)
                kts = [
                    qk_pool.tile([P, 8 * P], F16, tag=f"k{pc}", name=f"k{pc}")
                    for pc in range(2)
                ]
                qts = [
                    qk_pool.tile([P, CHUNK], F16, tag=f"q{pc}", name=f"q{pc}")
                    for pc in range(N_CHUNKS)
                ]
                vts = [
                    v_pool.tile([P, 8 * (HN + 1)], F16, tag=f"v{pc}", name=f"v{pc}")
                    for pc in range(2)
                ]
                nc.sync.dma_start(kts[0][:], kT_d[slot][:, 0:1024])
                nc.sync.dma_start(qts[7][:], qT_d[slot][:, 7 * CHUNK : 8 * CHUNK])
                nc.sync.dma_start(kts[1][:], kT_d[slot][:, 1024:2048])
                for pc in range(2):
                    nc.sync.dma_start(
                        vts[pc][:],
                        v_d[slot][:, pc * 8 * (HN + 1) : (pc + 1) * 8 * (HN + 1)],
                    )
                for pc in range(N_CHUNKS - 2, -1, -1):
                    nc.sync.dma_start(
                        qts[pc][:], qT_d[slot][:, pc * CHUNK : (pc + 1) * CHUNK]
                    )
                kslice = lambda j: kts[j // 8][:, (j % 8) * P : (j % 8 + 1) * P]
                qchunk = lambda ci: qts[ci][:]
                vslice = lambda j: vts[j // 8][
                    :, (j % 8) * (HN + 1) : (j % 8 + 1) * (HN + 1)
                ]
            else:
                kt = qk_pool.tile([P, SK], F16, tag="k")
                nc.sync.dma_start(kt[:], kT_d[slot])
                qt = qk_pool.tile([P, SQ], F16, tag="q")
                nc.sync.dma_start(qt[:], qT_d[slot])
                vt = v_pool.tile([P, N_SK_TILES * (HN + 1)], F16, tag="v")
                nc.sync.dma_start(vt[:], v_d[slot])
                kslice = lambda j: kt[:, j * P : (j + 1) * P]
                qchunk = lambda ci: qt[:, ci * CHUNK : (ci + 1) * CHUNK]
                vslice = lambda j: vt[:, j * (HN + 1) : (j + 1) * (HN + 1)]
            return kslice, qchunk, vslice

        # ---- emit one chunk's QK + exp; return et lookup ----------------
        def emit_qk(slot_io, ci):
            kslice, qchunk, _ = slot_io
            js = list(range(2 * ci + 2))     # ascending; diagonal j last
            diag = 2 * ci + 1
            exp_tiles = {}
            for g0 in range(0, len(js), GROUP):
                gjs = js[g0 : g0 + GROUP]
                sc = sc_ps.tile([P, GROUP * CHUNK], F32, tag="scores")
                gw = 0
                for k_idx, j in enumerate(gjs):
                    co = k_idx * CHUNK
                    w = P if j == diag else CHUNK
                    nc.tensor.matmul(
                        sc[:, co : co + w], kslice(j), qchunk(ci)[:, 0:w],
                        start=True, stop=True,
                    )
                    gw = co + w
                if diag in gjs:
                    # both triangular tiles sit on 256 contiguous cols:
                    # (second half of j=2ci) ++ (first half of j=2ci+1)
                    moff = gjs.index(2 * ci) * CHUNK + P
                    nc.vector.tensor_add(
                        sc[:, moff : moff + 2 * P],
                        sc[:, moff : moff + 2 * P],
                        tri_sb[:],
                    )
                et = e_pool.tile([P, GROUP * CHUNK], F16, tag="expT")
                nc.scalar.activation(
                    et[:, :gw], sc[:, :gw],
                    mybir.ActivationFunctionType.Exp,
                    scale=inv_norm,
                )
                for k_idx, j in enumerate(gjs):
                    exp_tiles[j] = (et, k_idx * CHUNK)
            return exp_tiles

        # ---- emit one chunk's PV + normalize + (maybe) out DMA ----------
        def emit_pv(slot, slot_io, ci, exp_tiles, oq_tiles, done_quarters):
            _, _, vslice = slot_io
            # one PSUM tile holds both context vectors of the chunk:
            # i_lo at cols [0,129), i_hi at cols [129,258)
            cx = cx_ps.tile([P, 2 * (HN + 1)], F32, tag="ctx")
            for i in (2 * ci + 1, 2 * ci):   # i_hi (first half of chunk), i_lo
                off = 0 if i == 2 * ci + 1 else P
                base = (HN + 1) if i == 2 * ci + 1 else 0
                pv_js = list(range(i + 1))
                for idx, j in enumerate(pv_js):
                    et, co = exp_tiles[j]
                    nc.tensor.matmul(
                        cx[:, base : base + HN + 1],
                        et[:, co + off : co + off + P], vslice(j),
                        start=(idx == 0), stop=(idx == len(pv_js) - 1),
                    )
            recip = r_pool.tile([P, 2], F32, tag="recip")
            nc.vector.reciprocal(
                recip[:], cx[:, HN : 2 * HN + 2 : HN + 1]
            )
            qt_idx = (2 * ci) // 4
            if qt_idx not in oq_tiles:
                oq_tiles[qt_idx] = o_pool.tile(
                    [P, 4 * HN], F32, tag="oq", name="oq"
                )
            ot = oq_tiles[qt_idx]
            col = (2 * ci % 4) * HN          # i_lo column; i_hi is the next one
            nc.vector.tensor_mul(
                ot[:, col : col + 2 * HN].rearrange("p (s c) -> p s c", s=2),
                cx[:].rearrange("p (s c) -> p s c", s=2)[:, :, 0:HN],
                recip[:].rearrange("p (s c) -> p s c", c=1).broadcast_to(
                    [P, 2, HN]
                ),
            )
            done_quarters.setdefault(qt_idx, set()).add(ci)
            if len(done_quarters[qt_idx]) == 2:
                nc.sync.dma_start(out_d[slot, qt_idx], oq_tiles[qt_idx][:])

        # ---- main schedule: PV pipelined one chunk behind QK ------------
        pending = None  # (slot, slot_io, ci, exp_tiles, oq_tiles, done_quarters)
        slot_state = {}
        for slot in range(SLOTS_PER_CORE):
            slot_io = load_slot(slot)
            order = range(N_CHUNKS - 1, -1, -1)
            slot_state[slot] = ({}, {})  # oq_tiles, done_quarters
            for ci in order:
                exp_tiles = emit_qk(slot_io, ci)
                if pending is not None:
                    emit_pv(*pending)
                oq, dq = slot_state[slot]
                pending = (slot, slot_io, ci, exp_tiles, oq, dq)
        emit_pv(*pending)

    nc.compile()
    return nc


_cache = {}


def _get_program(mask: np.ndarray):
    # this kernel is specialized to the standard causal mask
    m = np.asarray(mask)
    causal = np.triu(np.ones((SQ, SK), dtype=bool), k=1)
    for b in range(B):
        if not np.array_equal(m[b, 0], causal):
            raise ValueError("kernel specialized to causal attention mask")
    if "nc" not in _cache:
        _cache["nc"] = _build_program()
    return _cache["nc"]


def _core_slots(c):
    return [(0, 2 * c), (0, 2 * c + 1), (1, 2 * c), (1, 2 * c + 1)]


def prepare(query_layer, key_layer, value_layer, attention_mask):
    q = np.asarray(query_layer)
    k = np.asarray(key_layer)
    v = np.asarray(value_layer)
    nc = _get_program(np.asarray(attention_mask))

    # qT with the two 128-col tiles of each 256 chunk swapped:
    # sbuf layout col (256*ci + [0..255]) = sq (256*ci + [128..255, 0..127])
    q16 = q.astype(np.float16)                      # [SQ, B, NP, HN]
    qv = q16.reshape(N_CHUNKS, 2, P, B, NP, HN)[:, ::-1]   # swap tile pairs
    qT_all = np.ascontiguousarray(qv.transpose(3, 4, 5, 0, 1, 2)).reshape(
        B, NP, HN, SQ
    )
    k16 = k.astype(np.float16)
    kT_all = np.ascontiguousarray(k16.transpose(1, 2, 3, 0))  # [B, NP, HN, SK]

    v5 = v.reshape(N_SK_TILES, P, B, NP, HN).transpose(2, 3, 1, 0, 4)
    v_aug_all = np.empty((B, NP, P, N_SK_TILES, HN + 1), dtype=np.float16)
    v_aug_all[..., :HN] = v5
    v_aug_all[..., HN] = 1.0
    v_aug_all = v_aug_all.reshape(B, NP, P, N_SK_TILES * (HN + 1))

    # additive triangular tile (scoresT orientation): mask where sq < sk
    tri1 = np.where(
        np.arange(P)[None, :] < np.arange(P)[:, None], np.float32(NEG), np.float32(0)
    )
    tri = np.concatenate([tri1, tri1], axis=1)      # [128, 256]

    in_maps = []
    for c in range(N_CORES):
        slots = _core_slots(c)
        im = {
            "qT": np.ascontiguousarray(np.stack([qT_all[b, n] for b, n in slots])),
            "kT": np.ascontiguousarray(np.stack([kT_all[b, n] for b, n in slots])),
            "v_aug": np.ascontiguousarray(
                np.stack([v_aug_all[b, n] for b, n in slots])
            ),
            "tri": tri,
        }
        in_maps.append(im)
    return nc, in_maps


def assemble(results):
    """Gather per-core 'out' arrays into the full [SQ, B, NP*HN] output."""
    full = np.empty((SQ, B, NP * HN), dtype=np.float32)
    for c in range(N_CORES):
        o = results[c]["out"]  # [4, 4, 128, 512]
        for s, (b, n) in enumerate(_core_slots(c)):
            ctx = (
                o[s].reshape(4, P, 4, HN).transpose(0, 2, 1, 3).reshape(SQ, HN)
            )
            full[:, b, n * HN : (n + 1) * HN] = ctx
    return full


def kernel(query_layer, key_layer, value_layer, attention_mask):
    from concourse.bass_utils import run_bass_kernel_spmd

    nc, in_maps = prepare(query_layer, key_layer, value_layer, attention_mask)
    res = run_bass_kernel_spmd(nc, in_maps, list(range(N_CORES)))
    return assemble(res.results)


# revision 12
# speedup vs baseline: 476.1188x; 1.0262x over previous
"""Causal multi-head attention forward on 8 Trainium2 NeuronCores.

Problem: nn_CoreAttention (SQ=SK=2048, B=2, NP=16 heads, HN=128, fp32).

Sharding: 32 (batch, head) pairs split 4 per core (tensor-parallel over
heads, data-parallel over batch). No collectives.

Per (b, n) pair, in transposed score orientation (sk on partitions):
    scoresT[sk, sq] = (K Q^T)                 (PE matmul, fp16 in, hn contracted)
    expT = exp(scoresT * 1/sqrt(HN) + mask)   (ScalarE, fp16 out)
    ctx_aug[sq, hn+1] = expT^T @ [V | 1]      (PE matmul, sk contracted;
                                               col hn holds the softmax denom)
    ctx = ctx_aug[:, :hn] * 1/ctx_aug[:, hn]  (DVE reciprocal + scale)

v2 structure (vs the v1 baseline):
  - q/k cast to fp16 on host; no on-device casts; FWL stays enabled.
  - sq chunks of 256 with the two 128-tiles SWAPPED (host pre-swap), so the
    diagonal j-tile's masked half is the trailing half of the chunk: the
    diagonal QK matmul streams only 128 valid cols and exp skips the rest.
  - the two triangular tiles per chunk land on 256 contiguous score cols:
    one DVE add with a single [128,256] additive tile handles all masking.
  - PV is software-pipelined one chunk behind QK so the PE never stalls on
    the exp latency of the chunk it just produced.
  - slot 0 loads q/k/v in small pieces (fast start); the last slot walks
    chunks descending so the kernel tail is the smallest chunk.
  - outputs accumulate in SBUF and leave as 4 wide DMAs per slot.
"""

import math
import numpy as np
from contextlib import ExitStack

import concourse.bacc as bacc
import concourse.tile as tile
from concourse import mybir

SQ, SK, B, NP, HN = 2048, 2048, 2, 16, 128
N_CORES = 8
SLOTS_PER_CORE = 4
P = 128
CHUNK = 256
N_CHUNKS = SQ // CHUNK      # 8
N_SK_TILES = SK // P        # 16
NEG = -60000.0

import os
GROUP = int(os.environ.get("ATT_GROUP", "6"))          # j-tiles per PSUM group
SC_BUFS = int(os.environ.get("ATT_SC_BUFS", "2"))
CX_BUFS = int(os.environ.get("ATT_CX_BUFS", "2"))
E_BUFS = int(os.environ.get("ATT_E_BUFS", "6"))

F32 = mybir.dt.float32
F16 = mybir.dt.float16


def _build_program():
    nc = bacc.Bacc()

    qT_d = nc.declare_dram_parameter("qT", [SLOTS_PER_CORE, P, SQ], F16, isOutput=False)
    kT_d = nc.declare_dram_parameter("kT", [SLOTS_PER_CORE, P, SK], F16, isOutput=False)
    v_d = nc.declare_dram_parameter(
        "v_aug", [SLOTS_PER_CORE, P, N_SK_TILES * (HN + 1)], F16, isOutput=False
    )
    # triT[p, c] = NEG if p < c else 0; ident2 = [I | I]
    triT_d = nc.declare_dram_parameter("triT", [P, P], F16, isOutput=False)
    id2_d = nc.declare_dram_parameter("ident2", [P, 2 * P], F16, isOutput=False)
    out_d = nc.declare_dram_parameter(
        "out", [SLOTS_PER_CORE, 4, P, 4 * HN], F32, isOutput=True
    )

    inv_norm = 1.0 / math.sqrt(HN)

    with tile.TileContext(nc) as tc, ExitStack() as ctx:
        qk_pool = ctx.enter_context(tc.tile_pool(name="qk", bufs=2))
        v_pool = ctx.enter_context(tc.tile_pool(name="v", bufs=2))
        m_pool = ctx.enter_context(tc.tile_pool(name="m", bufs=1))
        e_pool = ctx.enter_context(tc.tile_pool(name="e", bufs=E_BUFS))
        o_pool = ctx.enter_context(tc.tile_pool(name="o", bufs=3))
        r_pool = ctx.enter_context(tc.tile_pool(name="r", bufs=4))
        sc_ps = ctx.enter_context(tc.tile_pool(name="sc", bufs=SC_BUFS, space="PSUM"))
        cx_ps = ctx.enter_context(tc.tile_pool(name="cx", bufs=CX_BUFS, space="PSUM"))

        triT_sb = m_pool.tile([P, P], F16, tag="triT")
        nc.sync.dma_start(triT_sb[:], triT_d[:])
        id2_sb = m_pool.tile([P, 2 * P], F16, tag="id2")
        nc.sync.dma_start(id2_sb[:], id2_d[:])

        # touch Exp immediately so the ACT table loads during the initial DMAs
        warm_in = m_pool.tile([P, 1], F32, tag="warm_in")
        nc.vector.memset(warm_in[:], 0.0)
        warm_out = m_pool.tile([P, 1], F32, tag="warm_out")
        nc.scalar.activation(
            warm_out[:], warm_in[:], mybir.ActivationFunctionType.Exp
        )

        # ---- per-slot input loading -------------------------------------
        def load_slot(slot):
            """Returns (kslice, qchunk, vslice) accessor fns for this slot."""
            if slot == 0:
                # pieces, issued in first-use order (chunks run descending)
                kts = [
                    qk_pool.tile([P, 8 * P], F16, tag=f"k{pc}", name=f"k{pc}")
                    for pc in range(2)
                ]
                qts = [
                    qk_pool.tile([P, CHUNK], F16, tag=f"q{pc}", name=f"q{pc}")
                    for pc in range(N_CHUNKS)
                ]
                vts = [
                    v_pool.tile([P, 8 * (HN + 1)], F16, tag=f"v{pc}", name=f"v{pc}")
                    for pc in range(2)
                ]
                nc.sync.dma_start(kts[0][:], kT_d[slot][:, 0:1024])
                nc.sync.dma_start(qts[7][:], qT_d[slot][:, 7 * CHUNK : 8 * CHUNK])
                nc.sync.dma_start(kts[1][:], kT_d[slot][:, 1024:2048])
                for pc in range(2):
                    nc.sync.dma_start(
                        vts[pc][:],
                        v_d[slot][:, pc * 8 * (HN + 1) : (pc + 1) * 8 * (HN + 1)],
                    )
                for pc in range(N_CHUNKS - 2, -1, -1):
                    nc.sync.dma_start(
                        qts[pc][:], qT_d[slot][:, pc * CHUNK : (pc + 1) * CHUNK]
                    )
                kslice = lambda j: kts[j // 8][:, (j % 8) * P : (j % 8 + 1) * P]
                qchunk = lambda ci: qts[ci][:]
                vslice = lambda j: vts[j // 8][
                    :, (j % 8) * (HN + 1) : (j % 8 + 1) * (HN + 1)
                ]
            else:
                kt = qk_pool.tile([P, SK], F16, tag="k")
                nc.sync.dma_start(kt[:], kT_d[slot])
                qt = qk_pool.tile([P, SQ], F16, tag="q")
                nc.sync.dma_start(qt[:], qT_d[slot])
                vt = v_pool.tile([P, N_SK_TILES * (HN + 1)], F16, tag="v")
                nc.sync.dma_start(vt[:], v_d[slot])
                kslice = lambda j: kt[:, j * P : (j + 1) * P]
                qchunk = lambda ci: qt[:, ci * CHUNK : (ci + 1) * CHUNK]
                vslice = lambda j: vt[:, j * (HN + 1) : (j + 1) * (HN + 1)]
            return kslice, qchunk, vslice

        # ---- emit one chunk's QK + exp; return et lookup ----------------
        def emit_qk(slot_io, ci):
            kslice, qchunk, _ = slot_io
            js = list(range(2 * ci + 2))     # ascending; diagonal j last
            diag = 2 * ci + 1
            exp_tiles = {}
            for g0 in range(0, len(js), GROUP):
                gjs = js[g0 : g0 + GROUP]
                sc = sc_ps.tile([P, GROUP * CHUNK], F32, tag="scores")
                gw = 0
                for k_idx, j in enumerate(gjs):
                    co = k_idx * CHUNK
                    w = P if j == diag else CHUNK
                    nc.tensor.matmul(
                        sc[:, co : co + w], kslice(j), qchunk(ci)[:, 0:w],
                        start=True, stop=True,
                    )
                    gw = co + w
                    # causal mask on the PE: sc[m, n] += triT[n%128, m].
                    # Must directly follow its QK matmul — start=False
                    # continues only the most recent accumulation group.
                    if j == diag:
                        nc.tensor.matmul(
                            sc[:, co : co + P], triT_sb[:], id2_sb[:, 0:P],
                            start=False, stop=True,
                        )
                    elif j == diag - 1:
                        nc.tensor.matmul(
                            sc[:, co + P : co + 2 * P], triT_sb[:], id2_sb[:, 0:P],
                            start=False, stop=True,
                        )
                et = e_pool.tile([P, GROUP * CHUNK], F16, tag="expT")
                nc.scalar.activation(
                    et[:, :gw], sc[:, :gw],
                    mybir.ActivationFunctionType.Exp,
                    scale=inv_norm,
                )
                for k_idx, j in enumerate(gjs):
                    exp_tiles[j] = (et, k_idx * CHUNK)
            return exp_tiles

        # ---- emit one chunk's PV + normalize + (maybe) out DMA ----------
        def emit_pv(slot, slot_io, ci, exp_tiles, oq_tiles, done_quarters):
            _, _, vslice = slot_io
            # one PSUM tile holds both context vectors of the chunk:
            # i_lo at cols [0,129), i_hi at cols [129,258)
            cx = cx_ps.tile([P, 2 * (HN + 1)], F32, tag="ctx")
            for i in (2 * ci + 1, 2 * ci):   # i_hi (first half of chunk), i_lo
                off = 0 if i == 2 * ci + 1 else P
                base = (HN + 1) if i == 2 * ci + 1 else 0
                pv_js = list(range(i + 1))
                for idx, j in enumerate(pv_js):
                    et, co = exp_tiles[j]
                    nc.tensor.matmul(
                        cx[:, base : base + HN + 1],
                        et[:, co + off : co + off + P], vslice(j),
                        start=(idx == 0), stop=(idx == len(pv_js) - 1),
                    )
            recip = r_pool.tile([P, 2], F32, tag="recip")
            nc.vector.reciprocal(
                recip[:], cx[:, HN : 2 * HN + 2 : HN + 1]
            )
            qt_idx = (2 * ci) // 4
            if qt_idx not in oq_tiles:
                oq_tiles[qt_idx] = o_pool.tile(
                    [P, 4 * HN], F32, tag="oq", name="oq"
                )
            ot = oq_tiles[qt_idx]
            col = (2 * ci % 4) * HN          # i_lo column; i_hi is the next one
            nc.vector.tensor_mul(
                ot[:, col : col + 2 * HN].rearrange("p (s c) -> p s c", s=2),
                cx[:].rearrange("p (s c) -> p s c", s=2)[:, :, 0:HN],
                recip[:].rearrange("p (s c) -> p s c", c=1).broadcast_to(
                    [P, 2, HN]
                ),
            )
            done_quarters.setdefault(qt_idx, set()).add(ci)
            if len(done_quarters[qt_idx]) == 2:
                nc.sync.dma_start(out_d[slot, qt_idx], oq_tiles[qt_idx][:])

        # ---- main schedule: PV pipelined one chunk behind QK ------------
        pending = None  # (slot, slot_io, ci, exp_tiles, oq_tiles, done_quarters)
        slot_state = {}
        for slot in range(SLOTS_PER_CORE):
            slot_io = load_slot(slot)
            order = range(N_CHUNKS - 1, -1, -1)
            slot_state[slot] = ({}, {})  # oq_tiles, done_quarters
            for ci in order:
                exp_tiles = emit_qk(slot_io, ci)
                if pending is not None:
                    emit_pv(*pending)
                oq, dq = slot_state[slot]
                pending = (slot, slot_io, ci, exp_tiles, oq, dq)
        emit_pv(*pending)

    nc.compile()
    return nc


_cache = {}


def _get_program(mask: np.ndarray):
    # this kernel is specialized to the standard causal mask
    m = np.asarray(mask)
    causal = np.triu(np.ones((SQ, SK), dtype=bool), k=1)
    for b in range(B):
        if not np.array_equal(m[b, 0], causal):
            raise ValueError("kernel specialized to causal attention mask")
    if "nc" not in _cache:
        _cache["nc"] = _build_program()
    return _cache["nc"]


def _core_slots(c):
    return [(0, 2 * c), (0, 2 * c + 1), (1, 2 * c), (1, 2 * c + 1)]


def prepare(query_layer, key_layer, value_layer, attention_mask):
    q = np.asarray(query_layer)
    k = np.asarray(key_layer)
    v = np.asarray(value_layer)
    nc = _get_program(np.asarray(attention_mask))

    # qT with the two 128-col tiles of each 256 chunk swapped:
    # sbuf layout col (256*ci + [0..255]) = sq (256*ci + [128..255, 0..127])
    q16 = q.astype(np.float16)                      # [SQ, B, NP, HN]
    qv = q16.reshape(N_CHUNKS, 2, P, B, NP, HN)[:, ::-1]   # swap tile pairs
    qT_all = np.ascontiguousarray(qv.transpose(3, 4, 5, 0, 1, 2)).reshape(
        B, NP, HN, SQ
    )
    k16 = k.astype(np.float16)
    kT_all = np.ascontiguousarray(k16.transpose(1, 2, 3, 0))  # [B, NP, HN, SK]

    v5 = v.reshape(N_SK_TILES, P, B, NP, HN).transpose(2, 3, 1, 0, 4)
    v_aug_all = np.empty((B, NP, P, N_SK_TILES, HN + 1), dtype=np.float16)
    v_aug_all[..., :HN] = v5
    v_aug_all[..., HN] = 1.0
    v_aug_all = v_aug_all.reshape(B, NP, P, N_SK_TILES * (HN + 1))

    # mask-matmul constants: sc[m, n] += sum_p triT[p, m] * ident2[p, n]
    #   = triT[n%128, m]  which must be NEG where (n%128) < m
    triT = np.where(
        np.arange(P)[:, None] < np.arange(P)[None, :], NEG, 0.0
    ).astype(np.float16)                            # triT[p, c] = NEG if p < c
    ident2 = np.concatenate([np.eye(P), np.eye(P)], axis=1).astype(np.float16)

    in_maps = []
    for c in range(N_CORES):
        slots = _core_slots(c)
        im = {
            "qT": np.ascontiguousarray(np.stack([qT_all[b, n] for b, n in slots])),
            "kT": np.ascontiguousarray(np.stack([kT_all[b, n] for b, n in slots])),
            "v_aug": np.ascontiguousarray(
                np.stack([v_aug_all[b, n] for b, n in slots])
            ),
            "triT": triT,
            "ident2": ident2,
        }
        in_maps.append(im)
    return nc, in_maps


def assemble(results):
    """Gather per-core 'out' arrays into the full [SQ, B, NP*HN] output."""
    full = np.empty((SQ, B, NP * HN), dtype=np.float32)
    for c in range(N_CORES):
        o = results[c]["out"]  # [4, 4, 128, 512]
        for s, (b, n) in enumerate(_core_slots(c)):
            ctx = (
                o[s].reshape(4, P, 4, HN).transpose(0, 2, 1, 3).reshape(SQ, HN)
            )
            full[:, b, n * HN : (n + 1) * HN] = ctx
    return full


def kernel(query_layer, key_layer, value_layer, attention_mask):
    from concourse.bass_utils import run_bass_kernel_spmd

    nc, in_maps = prepare(query_layer, key_layer, value_layer, attention_mask)
    res = run_bass_kernel_spmd(nc, in_maps, list(range(N_CORES)))
    return assemble(res.results)
